# revision 8
# baseline (speedup 1.0000x reference)
"""Trainium2 Bass kernel for chunked-prefill GQA attention with KV cache.

Problem (hardcoded shapes): N=2048 new queries, 32 q-heads / 8 kv-heads (GQA),
head_dim=128, kv cache pre-filled with 2048 tokens, new k/v appended at slots
2048..4095, offset-causal mask, softmax, out = attn @ v.

Sharding: tensor-parallel over heads. Core g handles kv-head g and q-heads
4g..4g+3. Embarrassingly parallel; no collectives.

Per-core kernel layout (all matmuls fp16, fp32 PSUM accumulate):
  - Q^T [128=hd, 2048] per head and K^T [128=hd, 4096] via host-cast fp16 +
    DMA-transpose (split into chunks so compute starts early). fp16 (not
    bf16): same PE speed, 8x lower quantization noise.
  - V natural [128=key, kb, 128+1] with a ones column; the PV matmul then
    yields both out-rows and the softmax denominator in one accumulation.
  - Scores computed transposed, S^T [128 keys, QCW queries] per key block;
    causal handled by block skipping + static multiplicative fp16 masks on
    the diagonal blocks (the 2 diagonal blocks form their own small final
    batch per strip to keep the strip-end exp->mask->PV chain short).
  - exp() is SPLIT across two engines (it is the serial bottleneck at
    ~1 elem/cycle/lane): the ACT engine runs the true exp LUT on 64% of the
    key blocks; the DVE engine covers DVE_FRAC=36% with a two-instruction
    Schraudolph pipeline: (1) tensor_scalar computes fp16 BITS of
    2^(t+BETA) as int16 round(x*A + B), bitcast to fp16; (2) a custom DVE
    op (registered at import into concourse.dve_ops.OPS) extracts the
    mantissa m with bitwise AND/OR (the OR mask doubles as the hardware One
    constant) and multiplies by the quadratic PC2*m^2+PC1*m+1, fitted
    jointly with BETA, cutting the bit-trick error from ~3.3% to ~0.6%.
    Final rel err 2.0e-3 (vs 2e-2 budget). NOTE: a [P,1] Src1 operand in a
    custom DVE op crashes this device; the spec must avoid Src1.
  - PSUM: 3 score buffers (KBATCH=4 blocks x QCW=256 = 2 banks each) + one
    accumulator bank per 128-query block. QK runs QK_AHEAD=2 batches ahead
    of exp so the PE never starves; pt (exp output) is 4-deep buffered.
    Two accumulation groups must NOT share a PSUM bank (start=True resets
    the whole bank, verified in CoreSim).
  - Both sq out-accumulators live in ONE 2-bank PSUM tile (bank-aligned
    via the 512-fp32 dim-1 stride) so a single reciprocal reads both
    denominators; the final diagonal block's sq0 PV (all-zero after the
    mask) is skipped. Numerically identical to the split form.
  - Tuning notes (HW-measured, rep-delta; device alternates fast/slow
    phases worth ~+/-15us): baseline all-ACT bf16 KBATCH=6 was 294us.
    KBATCH=4 + 3 sc buffers + lookahead + 36% DVE offload: 224us at
    1.77e-2 err (no polish, bf16). This config (fp16 + polish + merged
    outs): ~252-268us at 2.0e-3 err. Deviations measured WORSE on HW:
    DVE_SPLIT=0, out-of-place polish, masks on gpsimd, epilogue mul on ACT
    (ACT-from-PSUM per-call overhead), FRAC 0.26/0.30/0.40, ACT_SPLIT=2.
"""

import math

import numpy as np

N_Q = 2048
CHUNK_START = 2048
T_KEYS = 4096
H = 32
KVH = 8
HQ = H // KVH  # q heads per core
HD = 128
SCALE = 1.0 / math.sqrt(HD)
N_CORES = 8

QCW = 256  # query-chunk width (moving free dim of the QK^T matmul)
KBATCH = 4  # key blocks per exp() batch (score tile = 2 PSUM banks)
KB = T_KEYS // 128  # 32 key blocks
VW = HD + 1  # V row width incl. ones column
K_CHUNKS = [16, 16]  # key-block chunking for K^T/V loads
PT_BUFS = 4
OSB_BUFS = 2
DEN_BUFS = 8
SC_BUFS = 3

# exp() split: DVE_FRAC of key blocks go to the DVE Schraudolph pipeline
DVE_FRAC = 0.26
DVE_SPLIT = 2  # key blocks per DVE exp instruction (fine-grained unblocking)
ACT_SPLIT = 0  # key blocks per ACT exp instruction (0 = whole batch)
POLISH_2X = True  # declare the polish op eligible for the DVE 2x perf mode
QK_AHEAD = 2  # how many batches ahead the QK matmuls run (<= SC_BUFS - 1)
POLISH_OOP = False  # polish writes a second tile instead of in-place
MASK_ON_GPS = True  # diagonal mask multiplies on the gpsimd engine
SKIP_DIAG_SQ0 = True  # last diag block's sq0 PV is all-zero after mask
POLISH_EVERY = 1  # polish 1-in-N DVE batches (raw Schraudolph otherwise)
# epilogue: one engine copy of [out|den] PSUM->SBUF; the divide runs on host.
EPI_ENGINE = "dve"  # "act" | "dve" (gpsimd cannot read PSUM on HW)

_LOG2E = 1.4426950408889634
_BETA = 0.5475  # pass1 exponent offset; the polish poly absorbs 2^-BETA
DVE_A = 1024.0 * _LOG2E * SCALE      # fp16 bits per unit raw score
DVE_B = 1024.0 * (15.0 + _BETA)
_SIGMA0 = 0.0573  # zero-centers the raw (unpolished) bit-trick error
DVE_B0 = 1024.0 * (15.0 - _SIGMA0)
# quadratic polish y = y1*(PC2*m^2 + PC1*m + 1), m = 1 + mantissa(y1);
# the constant term is the hardware One const (a [P,1] Src1 operand
# crashes the DVE on this runtime, so the spec must avoid Src1)
PC2, PC1 = 0.15876613, -0.47503846
ANDMASK_F = float(np.uint32(0x007FE000).view(np.float32))


def _register_polish():
    """Register the custom DVE polish op (idempotent per process)."""
    import concourse.dve_ops as dvo
    from concourse.dve_ops import (
        CUSTOM_DVE_SPECS,
        OPS,
        DveOp,
        DveOpSpec,
        _SUB_OPCODE_FOR_NAME,
    )
    from concourse.dve_spec import AluOp, Bin, C0, C1, C2, One, Spec, Src0, Src1, lower

    name = "EXP_POLISH1_ANT" + ("_2X" if POLISH_2X else "")
    if name in _SUB_OPCODE_FOR_NAME:
        return next(o for o in OPS if o.name == name)
    a = Bin(AluOp.BITWISE_AND, Src0, C0)
    m = Bin(AluOp.BITWISE_OR, a, One)
    body = Src0 * ((C2 * m + C1) * m + One)

    def ref(in0, in1, s0, s1, imm2):
        bits = np.asarray(in0, dtype=np.float32).view(np.uint32)
        mask = np.float32(s0).view(np.uint32)
        one = np.float32(1.0).view(np.uint32)
        mm = ((bits & mask) | one).view(np.float32)
        return in0 * ((np.float32(imm2) * mm + np.float32(s1)) * mm + 1.0)

    spec = Spec(body=body, reference=ref)
    opcode = dvo._CUSTOM_DVE_ROW_BASE + len(OPS)
    shas = {
        v: DveOpSpec(name=name, opcode=opcode, uops=lower(spec, ver=v),
                     rd1_en=False).sha(v)
        for v in ("v3", "v4")
    }
    op = DveOp(name, spec, subdim=False, uops_sha=shas,
               perf_en={"v3": True, "v4": True} if POLISH_2X else {})
    OPS.append(op)
    _SUB_OPCODE_FOR_NAME[name] = opcode
    CUSTOM_DVE_SPECS[name] = spec
    return op


def _build_nc(reps: int = 1):
    import concourse.bacc as bacc
    import concourse.mybir as mybir
    import concourse.tile as tile

    polish = _register_polish()

    fp32 = mybir.dt.float32
    f16 = mybir.dt.float16
    i16 = mybir.dt.int16

    nc = bacc.Bacc("TRN2", target_bir_lowering=False, debug=False,
                   num_devices=N_CORES)

    q_in = nc.dram_tensor("q", [N_Q, HQ, HD], f16, kind="ExternalInput")
    k_in = nc.dram_tensor("k", [T_KEYS, HD], f16, kind="ExternalInput")
    v_in = nc.dram_tensor("v", [T_KEYS, HD], f16, kind="ExternalInput")
    # out carries the ones-column denominator in col HD; host divides
    out = nc.dram_tensor("out", [N_Q, HQ, VW], fp32, kind="ExternalOutput")

    n_qc = N_Q // QCW
    chunk_of = {}  # kb -> (chunk index, offset within chunk)
    _kb = 0
    for ci, w in enumerate(K_CHUNKS):
        for o in range(w):
            chunk_of[_kb] = (ci, o)
            _kb += 1
    assert _kb == KB

    with tile.TileContext(nc) as tc:
        with (
            tc.tile_pool(name="dram", bufs=1, space="DRAM") as dram,
            tc.tile_pool(name="const", bufs=1) as const,
            tc.tile_pool(name="pt", bufs=PT_BUFS) as ptpool,
            tc.tile_pool(name="pt2", bufs=PT_BUFS) as pt2pool,
            tc.tile_pool(name="osb", bufs=OSB_BUFS) as opool,
            tc.tile_pool(name="den", bufs=DEN_BUFS) as denpool,
            tc.tile_pool(name="scps", bufs=SC_BUFS, space="PSUM") as scpool,
            tc.tile_pool(name="outps", bufs=1, space="PSUM") as outpspool,
        ):
            # ---- transposed operands straight from f16 DRAM inputs ----
            # order: first-needed first (kt0, qt0, v0 feed the first batches)
            kts, qts, vsbs = [], [], []
            kb0c = 0
            for c, w in enumerate(K_CHUNKS):
                r0, r1 = kb0c * 128, (kb0c + w) * 128
                kb0c += w
                ktc = const.tile([128, w * 128], f16, name=f"kt{c}")
                nc.sync.dma_start_transpose(ktc[:], k_in.ap()[r0:r1, :])
                kts.append(ktc)
                if c == 0:
                    qtc = const.tile([128, N_Q], f16, name="qt0")
                    nc.sync.dma_start_transpose(qtc[:], q_in.ap()[:, 0, :])
                    qts.append(qtc)
                # V natural layout with ones column: [key%128, kb, hd+1]
                vc = const.tile([128, w, VW], f16, name=f"v{c}")
                nc.gpsimd.dma_start(
                    vc[:, :, 0:HD],
                    v_in.ap()[r0:r1, :].rearrange("(kb p) d -> p kb d", p=128),
                )
                nc.vector.memset(vc[:, :, HD:VW], 1.0)
                vsbs.append(vc)
            for h in range(1, HQ):
                qtc = const.tile([128, N_Q], f16, name=f"qt{h}")
                nc.sync.dma_start_transpose(qtc[:], q_in.ap()[:, h, :])
                qts.append(qtc)

            def kt_sl(kb):
                ci, o = chunk_of[kb]
                return kts[ci][:, o * 128:(o + 1) * 128]

            def v_sl(kb):
                ci, o = chunk_of[kb]
                return vsbs[ci][:, o, :]

            # ---- causal masks: mask[j][r, c] = 1.0 if r <= c - 128*j ----
            masks = const.tile([128, QCW // 128, QCW], f16)
            nc.vector.memset(masks[:], 1.0)
            for j in range(QCW // 128):
                nc.gpsimd.affine_select(
                    out=masks[:, j, :],
                    in_=masks[:, j, :],
                    compare_op=mybir.AluOpType.is_ge,
                    fill=0.0,
                    base=-128 * j,
                    pattern=[[1, QCW]],
                    channel_multiplier=-1,
                )

            # flat batch schedule over (head, q-chunk, key-block batch);
            # each batch carries its exp engine ('act' or 'dve')
            raw = []
            for h in range(HQ):
                for qc in range(n_qc):
                    n_kb = min(KB,
                               (CHUNK_START + (qc + 1) * QCW - 1) // 128 + 1)
                    # the 2 diagonal (masked) blocks form their own small
                    # final batch: keeps the strip-end exp->mask->PV chain
                    # short and maximizes DVE-eligible full batches before it
                    n_body = n_kb - 2
                    # balanced call sizes (e.g. 20 -> 5+5+5+5, not 6+6+6+2):
                    # tiny remainder exp() calls can't cover the PE's
                    # turnaround for the next batch and stall the pipeline
                    n_calls = -(-n_body // KBATCH)
                    base, extra = divmod(n_body, n_calls)
                    kb0 = 0
                    for ci in range(n_calls):
                        bsz = base + (1 if ci < extra else 0)
                        raw.append((h, qc, kb0, bsz, n_kb))
                        kb0 += bsz
                    raw.append((h, qc, kb0, 2, n_kb))
            batches = []
            dve_blocks = tot_blocks = 0
            for h, qc, kb0, bsz, n_kb in raw:
                # diagonal (mask-needing) blocks start at kb = 16 + 2*qc
                eligible = kb0 + bsz <= (CHUNK_START + qc * QCW) // 128
                use_dve = (eligible and
                           dve_blocks + bsz <=
                           DVE_FRAC * (tot_blocks + bsz))
                if use_dve:
                    dve_blocks += bsz
                    n_dve = sum(1 for b_ in batches if b_[5] != "act")
                    eng = ("dve" if POLISH_EVERY <= 1
                           or n_dve % POLISH_EVERY == 0 else "dve_raw")
                else:
                    eng = "act"
                tot_blocks += bsz
                batches.append((h, qc, kb0, bsz, n_kb, eng))

            def body():
                outs = None
                sc_tiles = {}

                def emit_qk(bi):
                    h, qc, kb0, bsz, n_kb, _eng = batches[bi]
                    sc = scpool.tile([128, KBATCH, QCW], fp32,
                                     name="sc", tag="sc")
                    sc_tiles[bi] = sc
                    for b in range(bsz):
                        kb = kb0 + b
                        nc.tensor.matmul(
                            sc[:, b, :],
                            lhsT=kt_sl(kb),
                            rhs=qts[h][:, qc * QCW:(qc + 1) * QCW],
                            start=True, stop=True,
                        )

                # lookahead: keep QK_AHEAD batches of scores in flight so PE
                # always has QK work while the exp engines drain batch bi
                emit_qk(0)
                if QK_AHEAD >= 2 and len(batches) > 1:
                    emit_qk(1)
                for bi in range(len(batches)):
                    h, qc, kb0, bsz, n_kb, eng = batches[bi]
                    if kb0 == 0:
                        # one PSUM bank per accumulation group (groups
                        # sharing a bank corrupt each other); merged: one
                        # 2-bank tile, each group bank-aligned via the 512
                        # fp32 dim-1 stride
                        outs_t = outpspool.tile([128, 2, 512], fp32,
                                                tag="out", name="out")
                        outs = [outs_t[:, i, 0:VW]
                                for i in range(QCW // 128)]
                    sc = sc_tiles.pop(bi)
                    pt = ptpool.tile([128, KBATCH, QCW], f16,
                                     name="pt", tag="pt")
                    if eng in ("dve", "dve_raw"):
                        # Schraudolph bits (+ quadratic mantissa polish on
                        # 1-in-POLISH_EVERY batches; raw tiles use the
                        # zero-centered B with no BETA offset)
                        raw = pt
                        if POLISH_OOP and eng == "dve":
                            raw = pt2pool.tile([128, KBATCH, QCW], f16,
                                               name="pt2", tag="pt2")
                        bconst = DVE_B if eng == "dve" else DVE_B0
                        step = DVE_SPLIT if DVE_SPLIT else bsz
                        for b0 in range(0, bsz, step):
                            b1 = min(b0 + step, bsz)
                            nc.vector.tensor_scalar(
                                raw[:, b0:b1, :].bitcast(i16),
                                sc[:, b0:b1, :], DVE_A, bconst,
                                mybir.AluOpType.mult, mybir.AluOpType.add,
                            )
                            if eng == "dve":
                                nc.vector._custom_dve(
                                    polish,
                                    out=pt[:, b0:b1, :],
                                    in0=raw[:, b0:b1, :],
                                    s0=ANDMASK_F, s1=PC1, imm2=PC2,
                                )
                    else:
                        step = ACT_SPLIT if ACT_SPLIT else bsz
                        for b0 in range(0, bsz, step):
                            b1 = min(b0 + step, bsz)
                            nc.scalar.activation(
                                pt[:, b0:b1, :], sc[:, b0:b1, :],
                                mybir.ActivationFunctionType.Exp,
                                scale=SCALE,
                            )
                    if bi + QK_AHEAD < len(batches):
                        emit_qk(bi + QK_AHEAD)
                    for b in range(bsz):
                        kb = kb0 + b
                        off = CHUNK_START + qc * QCW - kb * 128
                        if off < 128:  # diagonal block: apply mask
                            j = -off // 128 if off < 0 else 0
                            # the final diag block's left half is only read
                            # by the skipped sq0 PV: mask cols 128:256 only
                            c0 = 128 if (SKIP_DIAG_SQ0 and j == 1
                                         and kb == n_kb - 1) else 0
                            (nc.gpsimd if MASK_ON_GPS else nc.vector)\
                                .tensor_mul(
                                    pt[:, b, c0:QCW], pt[:, b, c0:QCW],
                                    masks[:, j, c0:QCW])
                    for b in range(bsz):
                        kb = kb0 + b
                        for sq in range(QCW // 128):
                            if (SKIP_DIAG_SQ0 and sq == 0
                                    and kb == n_kb - 1):
                                # mask zeroes pt[:, b, 0:128] for the final
                                # diagonal block: its sq0 PV adds zero
                                continue
                            stop = (kb == n_kb - 1 or
                                    (SKIP_DIAG_SQ0 and sq == 0
                                     and kb == n_kb - 2))
                            nc.tensor.matmul(
                                outs[sq],
                                lhsT=pt[:, b, sq * 128:(sq + 1) * 128],
                                rhs=v_sl(kb),
                                start=(kb == 0), stop=stop,
                            )
                    if kb0 + bsz >= n_kb:
                        # epilogue: single copy of [out|den] to SBUF;
                        # normalization happens on host (divide by col HD)
                        osb = opool.tile([128, QCW // 128, VW], fp32,
                                         name="osb", tag="osb")
                        epi = {"pool": nc.gpsimd, "act": None,
                               "dve": nc.vector}[EPI_ENGINE]
                        if EPI_ENGINE == "act":
                            nc.scalar.copy(osb[:], outs_t[:, :, 0:VW])
                        else:
                            epi.tensor_scalar_mul(osb[:],
                                                  outs_t[:, :, 0:VW], 1.0)
                        nc.sync.dma_start(
                            out.ap()[qc * QCW:(qc + 1) * QCW, h, :]
                               .rearrange("(s p) d -> p s d", p=128),
                            osb[:],
                        )

            if reps == 1:
                body()
            else:
                # timing-only loop; hint back-edge branch targets so the
                # IRAM refetch (~4us for >256-inst bodies) is prefetched
                with tc.For_i(0, reps, 1, hint_engines=(
                        mybir.EngineType.PE,
                        mybir.EngineType.Activation,
                        mybir.EngineType.DVE,
                        mybir.EngineType.SP,
                        mybir.EngineType.Pool)):
                    body()

    nc.compile()
    return nc


_NC_CACHE: dict = {}


def _get_nc(reps: int = 1):
    if reps not in _NC_CACHE:
        _NC_CACHE[reps] = _build_nc(reps)
    return _NC_CACHE[reps]


def _shard_inputs(q, k, v, k_cache, v_cache, slot_mapping, chunk_start):
    cs = int(chunk_start)
    n = q.shape[0]
    sm = np.asarray(slot_mapping)
    q = np.asarray(q, dtype=np.float32)
    k = np.asarray(k, dtype=np.float32)
    v = np.asarray(v, dtype=np.float32)
    k_cache = np.asarray(k_cache, dtype=np.float32)
    v_cache = np.asarray(v_cache, dtype=np.float32)

    if np.array_equal(sm, np.arange(n, dtype=sm.dtype) + cs):
        k_eff = np.concatenate([k_cache[:cs], k], axis=0)  # [T, KVH, HD]
        v_eff = np.concatenate([v_cache[:cs], v], axis=0)
    else:  # general path: honor arbitrary slot mappings
        kc = k_cache.copy()
        vc = v_cache.copy()
        kc[sm] = k
        vc[sm] = v
        k_eff = kc[:cs + n]
        v_eff = vc[:cs + n]

    k_eff = k_eff.astype(np.float16)
    v_eff = v_eff.astype(np.float16)
    q = q.astype(np.float16)

    in_maps = []
    for g in range(N_CORES):
        in_maps.append({
            "q": np.ascontiguousarray(q[:, g * HQ:(g + 1) * HQ, :]),
            "k": np.ascontiguousarray(k_eff[:, g, :]),
            "v": np.ascontiguousarray(v_eff[:, g, :]),
        })
    return in_maps


def kernel(q, k, v, k_cache, v_cache, slot_mapping, chunk_start, **_unused):
    from concourse import bass_utils

    in_maps = _shard_inputs(q, k, v, k_cache, v_cache, slot_mapping,
                            chunk_start)
    nc = _get_nc()
    res = bass_utils.run_bass_kernel_spmd(nc, in_maps,
                                          core_ids=list(range(N_CORES)))
    full = np.concatenate([res.results[g]["out"] for g in range(N_CORES)],
                          axis=1)  # [N, H, VW] with denominator in col HD
    return full[..., :HD] / full[..., HD:VW]



# revision 12
# speedup vs baseline: 2.3115x; 2.3115x over previous
"""Trainium2 Bass kernel for chunked-prefill GQA attention with KV cache.

Problem (hardcoded shapes): N=2048 new queries, 32 q-heads / 8 kv-heads (GQA),
head_dim=128, kv cache pre-filled with 2048 tokens, new k/v appended at slots
2048..4095, offset-causal mask, softmax, out = attn @ v.

Sharding: tensor-parallel over heads. Core g handles kv-head g and q-heads
4g..4g+3. Embarrassingly parallel; no collectives.

Per-core kernel layout (all matmuls fp16, fp32 PSUM accumulate):
  - Q^T [128=hd, 2048] per head and K^T [128=hd, 4096] via host-cast fp16 +
    DMA-transpose (split into chunks so compute starts early). fp16 (not
    bf16): same PE speed, 8x lower quantization noise.
  - V natural [128=key, kb, 128+1] with a ones column; the PV matmul then
    yields both out-rows and the softmax denominator in one accumulation.
  - Scores computed transposed, S^T [128 keys, QCW queries] per key block;
    causal handled by block skipping + static multiplicative fp16 masks on
    the diagonal blocks (the 2 diagonal blocks form their own small final
    batch per strip to keep the strip-end exp->mask->PV chain short).
  - exp() is SPLIT across two engines (it is the serial bottleneck at
    ~1 elem/cycle/lane): the ACT engine runs the true exp LUT on 64% of the
    key blocks; the DVE engine covers DVE_FRAC=36% with a two-instruction
    Schraudolph pipeline: (1) tensor_scalar computes fp16 BITS of
    2^(t+BETA) as int16 round(x*A + B), bitcast to fp16; (2) a custom DVE
    op (registered at import into concourse.dve_ops.OPS) extracts the
    mantissa m with bitwise AND/OR (the OR mask doubles as the hardware One
    constant) and multiplies by the quadratic PC2*m^2+PC1*m+1, fitted
    jointly with BETA, cutting the bit-trick error from ~3.3% to ~0.6%.
    Final rel err 2.0e-3 (vs 2e-2 budget). NOTE: a [P,1] Src1 operand in a
    custom DVE op crashes this device; the spec must avoid Src1.
  - PSUM: 3 score buffers (KBATCH=4 blocks x QCW=256 = 2 banks each) + one
    accumulator bank per 128-query block. QK runs QK_AHEAD=2 batches ahead
    of exp so the PE never starves; pt (exp output) is 4-deep buffered.
    Two accumulation groups must NOT share a PSUM bank (start=True resets
    the whole bank, verified in CoreSim).
  - Both sq out-accumulators live in ONE 2-bank PSUM tile (bank-aligned
    via the 512-fp32 dim-1 stride) so a single reciprocal reads both
    denominators; the final diagonal block's sq0 PV (all-zero after the
    mask) is skipped. Numerically identical to the split form.
  - Tuning notes (HW-measured, rep-delta; device alternates fast/slow
    phases worth ~+/-15us): baseline all-ACT bf16 KBATCH=6 was 294us.
    KBATCH=4 + 3 sc buffers + lookahead + 36% DVE offload: 224us at
    1.77e-2 err (no polish, bf16). This config (fp16 + polish + merged
    outs): ~252-268us at 2.0e-3 err. Deviations measured WORSE on HW:
    DVE_SPLIT=0, out-of-place polish, masks on gpsimd, epilogue mul on ACT
    (ACT-from-PSUM per-call overhead), FRAC 0.26/0.30/0.40, ACT_SPLIT=2.
"""

import math

import numpy as np

N_Q = 2048
CHUNK_START = 2048
T_KEYS = 4096
H = 32
KVH = 8
HQ = H // KVH  # q heads per core
HD = 128
SCALE = 1.0 / math.sqrt(HD)
N_CORES = 8

QCW = 256  # query-chunk width (moving free dim of the QK^T matmul)
KBATCH = 4  # key blocks per exp() batch (score tile = 2 PSUM banks)
KB = T_KEYS // 128  # 32 key blocks
VW = HD + 1  # V row width incl. ones column
K_CHUNKS = [16, 16]  # key-block chunking for K^T/V loads
PT_BUFS = 4
OSB_BUFS = 2
DEN_BUFS = 8
SC_BUFS = 3

# exp() split: DVE_FRAC of key blocks go to the DVE Schraudolph pipeline
DVE_FRAC = 0.26
DVE_SPLIT = 2  # key blocks per DVE exp instruction (fine-grained unblocking)
ACT_SPLIT = 0  # key blocks per ACT exp instruction (0 = whole batch)
POLISH_2X = True  # declare the polish op eligible for the DVE 2x perf mode
QK_AHEAD = 2  # how many batches ahead the QK matmuls run (<= SC_BUFS - 1)
POLISH_OOP = False  # polish writes a second tile instead of in-place
MASK_ON_GPS = True  # diagonal mask multiplies on the gpsimd engine
SKIP_DIAG_SQ0 = True  # last diag block's sq0 PV is all-zero after mask
POLISH_EVERY = 1  # polish 1-in-N DVE batches (raw Schraudolph otherwise)
# epilogue: one engine copy of [out|den] PSUM->SBUF; the divide runs on host.
EPI_ENGINE = "dve"  # "act" | "dve" (gpsimd cannot read PSUM on HW)

_LOG2E = 1.4426950408889634
_BETA = 0.5475  # pass1 exponent offset; the polish poly absorbs 2^-BETA
DVE_A = 1024.0 * _LOG2E * SCALE      # fp16 bits per unit raw score
DVE_B = 1024.0 * (15.0 + _BETA)
_SIGMA0 = 0.0573  # zero-centers the raw (unpolished) bit-trick error
DVE_B0 = 1024.0 * (15.0 - _SIGMA0)
# quadratic polish y = y1*(PC2*m^2 + PC1*m + 1), m = 1 + mantissa(y1);
# the constant term is the hardware One const (a [P,1] Src1 operand
# crashes the DVE on this runtime, so the spec must avoid Src1)
PC2, PC1 = 0.15876613, -0.47503846
ANDMASK_F = float(np.uint32(0x007FE000).view(np.float32))


def _register_polish():
    """Register the custom DVE polish op (idempotent per process)."""
    import concourse.dve_ops as dvo
    from concourse.dve_ops import (
        CUSTOM_DVE_SPECS,
        OPS,
        DveOp,
        DveOpSpec,
        _SUB_OPCODE_FOR_NAME,
    )
    from concourse.dve_spec import AluOp, Bin, C0, C1, C2, One, Spec, Src0, Src1, lower

    name = "EXP_POLISH1_ANT" + ("_2X" if POLISH_2X else "")
    if name in _SUB_OPCODE_FOR_NAME:
        return next(o for o in OPS if o.name == name)
    a = Bin(AluOp.BITWISE_AND, Src0, C0)
    m = Bin(AluOp.BITWISE_OR, a, One)
    body = Src0 * ((C2 * m + C1) * m + One)

    def ref(in0, in1, s0, s1, imm2):
        bits = np.asarray(in0, dtype=np.float32).view(np.uint32)
        mask = np.float32(s0).view(np.uint32)
        one = np.float32(1.0).view(np.uint32)
        mm = ((bits & mask) | one).view(np.float32)
        return in0 * ((np.float32(imm2) * mm + np.float32(s1)) * mm + 1.0)

    spec = Spec(body=body, reference=ref)
    opcode = dvo._CUSTOM_DVE_ROW_BASE + len(OPS)
    shas = {
        v: DveOpSpec(name=name, opcode=opcode, uops=lower(spec, ver=v),
                     rd1_en=False).sha(v)
        for v in ("v3", "v4")
    }
    op = DveOp(name, spec, subdim=False, uops_sha=shas,
               perf_en={"v3": True, "v4": True} if POLISH_2X else {})
    OPS.append(op)
    _SUB_OPCODE_FOR_NAME[name] = opcode
    CUSTOM_DVE_SPECS[name] = spec
    return op


def _build_nc(reps: int = 1):
    import concourse.bacc as bacc
    import concourse.mybir as mybir
    import concourse.tile as tile

    polish = _register_polish()

    fp32 = mybir.dt.float32
    f16 = mybir.dt.float16
    i16 = mybir.dt.int16

    nc = bacc.Bacc("TRN2", target_bir_lowering=False, debug=False,
                   num_devices=N_CORES)

    q_in = nc.dram_tensor("q", [N_Q, HQ, HD], f16, kind="ExternalInput")
    k_in = nc.dram_tensor("k", [T_KEYS, HD], f16, kind="ExternalInput")
    v_in = nc.dram_tensor("v", [T_KEYS, HD], f16, kind="ExternalInput")
    # out carries the ones-column denominator in col HD; host divides
    out = nc.dram_tensor("out", [N_Q, HQ, VW], fp32, kind="ExternalOutput")

    n_qc = N_Q // QCW
    chunk_of = {}  # kb -> (chunk index, offset within chunk)
    _kb = 0
    for ci, w in enumerate(K_CHUNKS):
        for o in range(w):
            chunk_of[_kb] = (ci, o)
            _kb += 1
    assert _kb == KB

    with tile.TileContext(nc) as tc:
        with (
            tc.tile_pool(name="dram", bufs=1, space="DRAM") as dram,
            tc.tile_pool(name="const", bufs=1) as const,
            tc.tile_pool(name="pt", bufs=PT_BUFS) as ptpool,
            tc.tile_pool(name="pt2", bufs=PT_BUFS) as pt2pool,
            tc.tile_pool(name="osb", bufs=OSB_BUFS) as opool,
            tc.tile_pool(name="den", bufs=DEN_BUFS) as denpool,
            tc.tile_pool(name="scps", bufs=SC_BUFS, space="PSUM") as scpool,
            tc.tile_pool(name="outps", bufs=2, space="PSUM") as outpspool,
        ):
            # ---- transposed operands straight from f16 DRAM inputs ----
            # order: first-needed first (kt0, qt0, v0 feed the first batches)
            kts, qts, vsbs = [], [], []
            kb0c = 0
            for c, w in enumerate(K_CHUNKS):
                r0, r1 = kb0c * 128, (kb0c + w) * 128
                kb0c += w
                ktc = const.tile([128, w * 128], f16, name=f"kt{c}")
                nc.sync.dma_start_transpose(ktc[:], k_in.ap()[r0:r1, :])
                kts.append(ktc)
                if c == 0:
                    qtc = const.tile([128, N_Q], f16, name="qt0")
                    nc.sync.dma_start_transpose(qtc[:], q_in.ap()[:, 0, :])
                    qts.append(qtc)
                # V natural layout with ones column: [key%128, kb, hd+1]
                vc = const.tile([128, w, VW], f16, name=f"v{c}")
                nc.gpsimd.dma_start(
                    vc[:, :, 0:HD],
                    v_in.ap()[r0:r1, :].rearrange("(kb p) d -> p kb d", p=128),
                )
                nc.vector.memset(vc[:, :, HD:VW], 1.0)
                vsbs.append(vc)
            for h in range(1, HQ):
                qtc = const.tile([128, N_Q], f16, name=f"qt{h}")
                nc.sync.dma_start_transpose(qtc[:], q_in.ap()[:, h, :])
                qts.append(qtc)

            def kt_sl(kb):
                ci, o = chunk_of[kb]
                return kts[ci][:, o * 128:(o + 1) * 128]

            def v_sl(kb):
                ci, o = chunk_of[kb]
                return vsbs[ci][:, o, :]

            # ---- causal masks: mask[j][r, c] = 1.0 if r <= c - 128*j ----
            masks = const.tile([128, QCW // 128, QCW], f16)
            nc.vector.memset(masks[:], 1.0)
            for j in range(QCW // 128):
                nc.gpsimd.affine_select(
                    out=masks[:, j, :],
                    in_=masks[:, j, :],
                    compare_op=mybir.AluOpType.is_ge,
                    fill=0.0,
                    base=-128 * j,
                    pattern=[[1, QCW]],
                    channel_multiplier=-1,
                )

            # flat batch schedule over (head, q-chunk, key-block batch);
            # each batch carries its exp engine ('act' or 'dve')
            raw = []
            for h in range(HQ):
                for qc in range(n_qc):
                    n_kb = min(KB,
                               (CHUNK_START + (qc + 1) * QCW - 1) // 128 + 1)
                    # the 2 diagonal (masked) blocks form their own small
                    # final batch: keeps the strip-end exp->mask->PV chain
                    # short and maximizes DVE-eligible full batches before it
                    n_body = n_kb - 2
                    # balanced call sizes (e.g. 20 -> 5+5+5+5, not 6+6+6+2):
                    # tiny remainder exp() calls can't cover the PE's
                    # turnaround for the next batch and stall the pipeline
                    n_calls = -(-n_body // KBATCH)
                    base, extra = divmod(n_body, n_calls)
                    kb0 = 0
                    for ci in range(n_calls):
                        bsz = base + (1 if ci < extra else 0)
                        raw.append((h, qc, kb0, bsz, n_kb))
                        kb0 += bsz
                    raw.append((h, qc, kb0, 2, n_kb))
            batches = []
            dve_blocks = tot_blocks = 0
            for h, qc, kb0, bsz, n_kb in raw:
                # diagonal (mask-needing) blocks start at kb = 16 + 2*qc
                eligible = kb0 + bsz <= (CHUNK_START + qc * QCW) // 128
                use_dve = (eligible and
                           dve_blocks + bsz <=
                           DVE_FRAC * (tot_blocks + bsz))
                if use_dve:
                    dve_blocks += bsz
                    n_dve = sum(1 for b_ in batches if b_[5] != "act")
                    eng = ("dve" if POLISH_EVERY <= 1
                           or n_dve % POLISH_EVERY == 0 else "dve_raw")
                else:
                    eng = "act"
                tot_blocks += bsz
                batches.append((h, qc, kb0, bsz, n_kb, eng))

            def body():
                outs = None
                sc_tiles = {}

                def emit_qk(bi):
                    h, qc, kb0, bsz, n_kb, _eng = batches[bi]
                    sc = scpool.tile([128, KBATCH, QCW], fp32,
                                     name="sc", tag="sc")
                    sc_tiles[bi] = sc
                    for b in range(bsz):
                        kb = kb0 + b
                        nc.tensor.matmul(
                            sc[:, b, :],
                            lhsT=kt_sl(kb),
                            rhs=qts[h][:, qc * QCW:(qc + 1) * QCW],
                            start=True, stop=True,
                        )

                # lookahead: keep QK_AHEAD batches of scores in flight so PE
                # always has QK work while the exp engines drain batch bi
                emit_qk(0)
                if QK_AHEAD >= 2 and len(batches) > 1:
                    emit_qk(1)
                for bi in range(len(batches)):
                    h, qc, kb0, bsz, n_kb, eng = batches[bi]
                    if kb0 == 0:
                        # both sq accumulators share ONE PSUM bank: the
                        # first matmul's start=True zeroes the whole 2KB
                        # bank (ZERO_REGION granularity), then every other
                        # matmul accumulates with start=False into its own
                        # 129-col sub-range. Frees a bank so outs can be
                        # double-buffered (no strip-tail stall).
                        outs_t = outpspool.tile([128, 512], fp32,
                                                tag="out", name="out")
                        outs = [outs_t[:, i * VW:(i + 1) * VW]
                                for i in range(QCW // 128)]
                    sc = sc_tiles.pop(bi)
                    pt = ptpool.tile([128, KBATCH, QCW], f16,
                                     name="pt", tag="pt")
                    if eng in ("dve", "dve_raw"):
                        # Schraudolph bits (+ quadratic mantissa polish on
                        # 1-in-POLISH_EVERY batches; raw tiles use the
                        # zero-centered B with no BETA offset)
                        raw = pt
                        if POLISH_OOP and eng == "dve":
                            raw = pt2pool.tile([128, KBATCH, QCW], f16,
                                               name="pt2", tag="pt2")
                        bconst = DVE_B if eng == "dve" else DVE_B0
                        step = DVE_SPLIT if DVE_SPLIT else bsz
                        for b0 in range(0, bsz, step):
                            b1 = min(b0 + step, bsz)
                            nc.vector.tensor_scalar(
                                raw[:, b0:b1, :].bitcast(i16),
                                sc[:, b0:b1, :], DVE_A, bconst,
                                mybir.AluOpType.mult, mybir.AluOpType.add,
                            )
                            if eng == "dve":
                                nc.vector._custom_dve(
                                    polish,
                                    out=pt[:, b0:b1, :],
                                    in0=raw[:, b0:b1, :],
                                    s0=ANDMASK_F, s1=PC1, imm2=PC2,
                                )
                    else:
                        step = ACT_SPLIT if ACT_SPLIT else bsz
                        for b0 in range(0, bsz, step):
                            b1 = min(b0 + step, bsz)
                            nc.scalar.activation(
                                pt[:, b0:b1, :], sc[:, b0:b1, :],
                                mybir.ActivationFunctionType.Exp,
                                scale=SCALE,
                            )
                    if bi + QK_AHEAD < len(batches):
                        emit_qk(bi + QK_AHEAD)
                    for b in range(bsz):
                        kb = kb0 + b
                        off = CHUNK_START + qc * QCW - kb * 128
                        if off < 128:  # diagonal block: apply mask
                            j = -off // 128 if off < 0 else 0
                            # the final diag block's left half is only read
                            # by the skipped sq0 PV: mask cols 128:256 only
                            c0 = 128 if (SKIP_DIAG_SQ0 and j == 1
                                         and kb == n_kb - 1) else 0
                            (nc.gpsimd if MASK_ON_GPS else nc.vector)\
                                .tensor_mul(
                                    pt[:, b, c0:QCW], pt[:, b, c0:QCW],
                                    masks[:, j, c0:QCW])
                    for b in range(bsz):
                        kb = kb0 + b
                        for sq in range(QCW // 128):
                            if (SKIP_DIAG_SQ0 and sq == 0
                                    and kb == n_kb - 1):
                                # mask zeroes pt[:, b, 0:128] for the final
                                # diagonal block: its sq0 PV adds zero
                                continue
                            stop = (kb == n_kb - 1 or
                                    (SKIP_DIAG_SQ0 and sq == 0
                                     and kb == n_kb - 2))
                            # only the very first matmul into the shared
                            # bank uses start=True (zeroes the whole bank,
                            # covering both sq sub-ranges)
                            nc.tensor.matmul(
                                outs[sq],
                                lhsT=pt[:, b, sq * 128:(sq + 1) * 128],
                                rhs=v_sl(kb),
                                start=(kb == 0 and sq == 0), stop=stop,
                                skip_group_check=True,
                            )
                    if kb0 + bsz >= n_kb:
                        # epilogue: single copy of [out|den] to SBUF;
                        # normalization happens on host (divide by col HD)
                        osb = opool.tile([128, QCW // 128, VW], fp32,
                                         name="osb", tag="osb")
                        osrc = outs_t[:, 0:(QCW // 128) * VW].rearrange(
                            "p (s d) -> p s d", d=VW)
                        if EPI_ENGINE == "act":
                            nc.scalar.copy(osb[:], osrc)
                        else:
                            nc.vector.tensor_scalar_mul(osb[:], osrc, 1.0)
                        nc.sync.dma_start(
                            out.ap()[qc * QCW:(qc + 1) * QCW, h, :]
                               .rearrange("(s p) d -> p s d", p=128),
                            osb[:],
                        )

            if reps == 1:
                body()
            else:
                # timing-only loop; hint back-edge branch targets so the
                # IRAM refetch (~4us for >256-inst bodies) is prefetched
                with tc.For_i(0, reps, 1, hint_engines=(
                        mybir.EngineType.PE,
                        mybir.EngineType.Activation,
                        mybir.EngineType.DVE,
                        mybir.EngineType.SP,
                        mybir.EngineType.Pool)):
                    body()

    nc.compile()
    return nc


_NC_CACHE: dict = {}


def _get_nc(reps: int = 1):
    if reps not in _NC_CACHE:
        _NC_CACHE[reps] = _build_nc(reps)
    return _NC_CACHE[reps]


def _shard_inputs(q, k, v, k_cache, v_cache, slot_mapping, chunk_start):
    cs = int(chunk_start)
    n = q.shape[0]
    sm = np.asarray(slot_mapping)
    q = np.asarray(q, dtype=np.float32)
    k = np.asarray(k, dtype=np.float32)
    v = np.asarray(v, dtype=np.float32)
    k_cache = np.asarray(k_cache, dtype=np.float32)
    v_cache = np.asarray(v_cache, dtype=np.float32)

    if np.array_equal(sm, np.arange(n, dtype=sm.dtype) + cs):
        k_eff = np.concatenate([k_cache[:cs], k], axis=0)  # [T, KVH, HD]
        v_eff = np.concatenate([v_cache[:cs], v], axis=0)
    else:  # general path: honor arbitrary slot mappings
        kc = k_cache.copy()
        vc = v_cache.copy()
        kc[sm] = k
        vc[sm] = v
        k_eff = kc[:cs + n]
        v_eff = vc[:cs + n]

    k_eff = k_eff.astype(np.float16)
    v_eff = v_eff.astype(np.float16)
    q = q.astype(np.float16)

    in_maps = []
    for g in range(N_CORES):
        in_maps.append({
            "q": np.ascontiguousarray(q[:, g * HQ:(g + 1) * HQ, :]),
            "k": np.ascontiguousarray(k_eff[:, g, :]),
            "v": np.ascontiguousarray(v_eff[:, g, :]),
        })
    return in_maps


def kernel(q, k, v, k_cache, v_cache, slot_mapping, chunk_start, **_unused):
    from concourse import bass_utils

    in_maps = _shard_inputs(q, k, v, k_cache, v_cache, slot_mapping,
                            chunk_start)
    nc = _get_nc()
    res = bass_utils.run_bass_kernel_spmd(nc, in_maps,
                                          core_ids=list(range(N_CORES)))
    full = np.concatenate([res.results[g]["out"] for g in range(N_CORES)],
                          axis=1)  # [N, H, VW] with denominator in col HD
    return full[..., :HD] / full[..., HD:VW]

